# revision 37
# baseline (speedup 1.0000x reference)
"""Multi-head attention (B=2, N=2048, C=768, H=12) on 8 TRN2 NeuronCores.

Sharding: core c = 4*b + g handles batch b (data parallel) and heads
3g..3g+2 (tensor parallel on H). Each core computes its 3 heads end-to-end
plus the partial projection with its 192 rows of w_proj; the host sums the
4 partials per batch and adds b_proj. No cross-device communication.

Clock strategy: the PE reaches/holds 2.4 GHz only while its instruction
stream never waits on a semaphore; switching the PE tiling mode costs
~106ns per transition. Steady state is grouped so the PE always outpaces
ACT (the exp engine, ~1.04us per [128,1024] tile = the true floor) and
therefore never stalls:

  per 2-m-chunk group:  [scores m, m+1]   64-row-tile T0/T8 pairs (~432ns)
                        [attn@v m-3, m-2] full-K matmuls         (~930ns)
                        [one filler item]  full-K qk/v/proj piece (~300-600ns)
ACT does exp only; DVE does all PSUM evacuation + softmax normalization;
junk matmuls pad slots when no real filler remains.
"""

from collections import deque

import ml_dtypes
import numpy as np

import concourse.bass as bass
import concourse.mybir as mybir
import concourse.tile as tile
from concourse import bacc
from concourse.bass_utils import run_bass_kernel_spmd

F32 = mybir.dt.float32
F32R = mybir.dt.float32r
BF16 = mybir.dt.bfloat16
EXP = mybir.ActivationFunctionType.Exp
MULT = mybir.AluOpType.mult
ADD = mybir.AluOpType.add
I16 = mybir.dt.int16

B, N, C = 2, 2048, 768
H = 12
D = 64
HPC = 3  # heads per core
KC = 6  # contraction chunks of 128 over C
NB = 1024  # n-block for attention stage
MC = N // 128  # 16 m-chunks (context)
NCH = N // 128  # 16 output row chunks
SCALE = D ** -0.5
# Schraudolph-style integer exp constants for bf16 output bits
# (fold the 1/sqrt(D) attention scale into K1)
EXP_K1 = float(SCALE * 1.4426950408889634 * (1 << 7))
EXP_K2 = float((127 << 7) - 486411.0 / (1 << 16))
LAG = 4  # attn@v trails exp by this many m-chunks

_NC_CACHE = None


def build_nc():
    nc = bacc.Bacc("TRN2", target_bir_lowering=False, debug=False, num_devices=8)
    # host pre-packs every input into its exact SBUF layout so each DMA
    # moves long contiguous per-partition lines (4-24KB) at full HBM rate
    xt = nc.declare_dram_parameter("xt", [128, KC * N], BF16, isOutput=False)
    wqk = nc.declare_dram_parameter("wqk", [128, KC * HPC * 128], BF16, isOutput=False)
    wv = nc.declare_dram_parameter("wv", [128, KC * HPC * D], BF16, isOutput=False)
    wp = nc.declare_dram_parameter("wp", [HPC * D, C], BF16, isOutput=False)
    out = nc.declare_dram_parameter("out", [128, NCH * C], BF16, isOutput=True)

    with tile.TileContext(nc) as tc:
        with (
            tc.tile_pool(name="sb", bufs=1) as sb,
            tc.tile_pool(name="ps", bufs=1, space="PSUM") as ps,
            tc.tile_pool(name="drp", bufs=2, space="DRAM") as drp,
        ):
            # ---- input loads --------------------------------------------
            xtb = sb.tile([128, KC * N], BF16, tag="xtb")
            wqkb = sb.tile([128, KC * HPC * 128], BF16, tag="wqkb")
            wvb = sb.tile([128, KC * HPC * D], BF16, tag="wvb")
            # weights first (small), then x token-half by token-half so the
            # first attention block can start after ~half the load
            wp01 = sb.tile([128, C], BF16, tag="wp01")
            wp2 = sb.tile([64, C], BF16, tag="wp2")
            nc.sync.dma_start(wqkb[:], wqk[:])
            nc.scalar.dma_start(wvb[:], wv[:])
            for th in range(2):
                for kc in range(KC):
                    for pg in range(2):
                        sl = slice(pg * 64, (pg + 1) * 64)
                        eng = nc.sync if (kc + pg) % 2 else nc.scalar
                        eng.dma_start(
                            xtb[sl, kc * N + th * NB : kc * N + (th + 1) * NB],
                            xt[sl, kc * N + th * NB : kc * N + (th + 1) * NB],
                        )
            nc.sync.dma_start(wp01[:], wp[0:128, :])
            nc.sync.dma_start(wp2[:], wp[128 : HPC * D, :])

            junk = sb.tile([128, 512], BF16, tag="junk")
            nc.vector.memset(junk[:], 1.0)

            ones_f = sb.tile([128, MC], F32, tag="ones_f")
            nc.vector.memset(ones_f[:], 1.0)
            v_sb = sb.tile([128, HPC * MC * 65], BF16, tag="v")
            v4 = v_sb.rearrange("p (h m w) -> p h m w", h=HPC, m=MC)
            for h in range(HPC):
                nc.vector.tensor_copy(v4[:, h, :, 64], ones_f[:, :])

            qk_sb = [
                sb.tile([128, N], BF16, tag=f"qk{h}", name=f"qk{h}") for h in range(HPC)
            ]
            kq_sb = [
                sb.tile([128, N], BF16, tag=f"kq{h}", name=f"kq{h}") for h in range(HPC)
            ]
            stk = sb.tile([128, N], BF16, tag="stk")
            outT1 = sb.tile([64, N], BF16, tag="outT1")
            outT2 = sb.tile([64, N], BF16, tag="outT2")

            def sc_tile(name):
                return ps.tile([128, NB], F32, tag="sc", bufs=3, name=name)

            # ---- lead-in (full mode): ramp + qk head 0 ------------------
            for i in range(4):
                psw = sc_tile(f"junk{i}")
                nc.tensor.matmul(
                    psw[:, 0:512], junk[:, 0:128], junk[:], start=True, stop=True
                )
                nc.tensor.matmul(
                    psw[:, 512:1024], junk[:, 0:128], junk[:], start=True, stop=True
                )

            def emit_qk_half(h, half, psq, kc):
                hb = half * 1024
                for s in range(2):
                    nc.tensor.matmul(
                        psq[:, s * 512 : (s + 1) * 512],
                        wqkb[
                            :, kc * HPC * 128 + h * 128 : kc * HPC * 128 + (h + 1) * 128
                        ],
                        xtb[:, kc * N + hb + s * 512 : kc * N + hb + (s + 1) * 512],
                        start=(kc == 0),
                        stop=(kc == KC - 1),
                    )

            def emit_qk_tail(h, half, psq):
                hb = half * 1024
                nc.vector.tensor_copy(qk_sb[h][:, hb : hb + 1024], psq[:])
                eng = nc.scalar if h == 0 else nc.sync
                eng.dma_start(
                    kq_sb[h][0:64, hb : hb + 1024], qk_sb[h][64:128, hb : hb + 1024]
                )
                eng.dma_start(
                    kq_sb[h][64:128, hb : hb + 1024], qk_sb[h][0:64, hb : hb + 1024]
                )

            for half in range(2):
                psq = sc_tile(f"qk0h{half}")
                for kc in range(KC):
                    emit_qk_half(0, half, psq, kc)
                emit_qk_tail(0, half, psq)

            # ---- background PE work (full mode) -------------------------
            bg = deque()

            def v_item(m):
                def emit():
                    psv = sc_tile(f"v{m}")
                    for kc in range(KC):
                        nc.tensor.matmul(
                            psv[:, 0 : HPC * D],
                            xtb[:, kc * N + m * 128 : kc * N + (m + 1) * 128],
                            wvb[:, kc * HPC * D : (kc + 1) * HPC * D],
                            start=(kc == 0),
                            stop=(kc == KC - 1),
                        )
                    nc.vector.tensor_copy(
                        v4[:, :, m, 0:64],
                        psv[:, 0 : HPC * D].rearrange("p (h d) -> p h d", h=HPC),
                    )

                return emit

            def qk_item(h, half, state, step):
                # step 0..KC-1: one kc contraction step (2 matmuls ~432ns);
                # step KC: evacuation copy + partition swap (no PE work)
                def emit():
                    if step == 0:
                        state["ps"] = sc_tile(f"qk{h}h{half}")
                    if step < KC:
                        emit_qk_half(h, half, state["ps"], step)
                    else:
                        emit_qk_tail(h, half, state["ps"])

                return emit

            ob_state = {}

            def proj_item(k):
                def emit():
                    pp = sc_tile(f"pp{k}")
                    for sw, w in ((0, 512), (512, 256)):
                        nc.tensor.matmul(
                            pp[:, sw : sw + w],
                            stk[:, k * 128 : (k + 1) * 128],
                            wp01[:, sw : sw + w],
                            start=True,
                            stop=False,
                        )
                    for sw, w in ((0, 512), (512, 256)):
                        nc.tensor.matmul(
                            pp[:, sw : sw + w],
                            outT2[0:64, k * 128 : (k + 1) * 128],
                            wp2[:, sw : sw + w],
                            start=False,
                            stop=True,
                        )
                    if k % 2 == 0:
                        ob_state["t"] = sb.tile(
                            [128, 2 * C], BF16, tag="ob", bufs=2, name=f"ob{k}"
                        )
                    ob = ob_state["t"]
                    half = (k % 2) * C
                    if k >= 8:
                        # tail chunks: ACT is idle, split the evacuation
                        nc.vector.tensor_copy(ob[:, half : half + 384], pp[:, 0:384])
                        nc.scalar.copy(out=ob[:, half + 384 : half + C], in_=pp[:, 384:C])
                    else:
                        nc.vector.tensor_copy(ob[:, half : half + C], pp[:, 0:C])
                    if k % 2 == 1:
                        eng = nc.scalar if k >= 8 and (k // 2) % 2 else nc.sync
                        eng.dma_start(out[:, (k - 1) * C : (k + 1) * C], ob[:])

                return emit

            for m in range(MC):
                bg.append((640, v_item(m), False))
            for h in (1, 2):
                for half in range(2):
                    state = {}
                    for step in range(KC + 1):
                        bg.append(
                            (440 if step < KC else 0, qk_item(h, half, state, step), False)
                        )

            pad_idx = [0]

            def emit_pad(n_mm):
                psw = sc_tile(f"pad{pad_idx[0]}")
                pad_idx[0] += 1
                for s in range(n_mm):
                    nc.tensor.matmul(
                        psw[:, (s % 2) * 512 : (s % 2 + 1) * 512],
                        junk[:, 0:128],
                        junk[:],
                        start=True,
                        stop=True,
                    )

            def pump(budget, pad=False, allow_proj=True):
                # consume bg items until the PE-time budget is spent; if the
                # queue is dry, emit ONE junk pad tile to fill the remainder
                while bg and budget > 0:
                    if bg[0][2] and not allow_proj:
                        break
                    cost, emit, _ = bg.popleft()
                    emit()
                    budget -= cost
                if pad and budget > 200:
                    emit_pad(min(4, max(2, round(budget / 233))))

            # ---- attention: blocks software-pipelined across boundaries --
            # The last two oa groups + the normalization epilogue of block i
            # run during the first groups of block i+1 so ACT never starves
            # at a boundary waiting for the PE to sprint the drain.
            carry = deque()

            def make_block(h, nb):
                state = {"ex": {}, "oa": None}

                def escores(m):
                    sc = sc_tile(f"sc{h}_{nb}_{m}")
                    nc.tensor.matmul(
                        sc[:, 0:512],
                        kq_sb[h][0:64, m * 128 : (m + 1) * 128],
                        qk_sb[h][0:64, nb * NB : nb * NB + 512],
                        start=True,
                        stop=True,
                        tile_position=(0, 0),
                    )
                    nc.tensor.matmul(
                        sc[:, 512:1024],
                        qk_sb[h][64:128, m * 128 : (m + 1) * 128],
                        kq_sb[h][64:128, nb * NB + 512 : nb * NB + 1024],
                        start=True,
                        stop=True,
                        tile_position=(64, 0),
                    )
                    ex = sb.tile([128, NB], BF16, tag="ex", bufs=9)
                    if m % 5 == 4:
                        # offload to DVE: exp(x) ~ bitcast(int32(x*K1 + K2));
                        # end-to-end rel err stays < 1e-2 (verified in sim)
                        nc.vector.tensor_scalar(
                            out=ex[:].bitcast(I16),
                            in0=sc[:],
                            scalar1=EXP_K1,
                            scalar2=EXP_K2,
                            op0=MULT,
                            op1=ADD,
                        )
                    else:
                        nc.scalar.activation(ex[:], sc[:], EXP, scale=SCALE)
                    state["ex"][m] = ex

                def eoa(m):
                    exm = state["ex"].pop(m)
                    for s in range(2):
                        nc.tensor.matmul(
                            state["oa"][:, s * 512 : (s + 1) * 512],
                            v4[:, h, m, :],
                            exm[:, s * 512 : (s + 1) * 512],
                            start=(m == 0),
                            stop=(m == MC - 1),
                        )

                def epilogue():
                    final = h == HPC - 1 and nb == 1
                    oa = state["oa"]
                    cs = sb.tile([1, NB], F32, tag="cs", bufs=2)
                    nc.vector.tensor_copy(cs[:], oa[64:65, :])
                    if final:
                        osb = oa[0:64, :]
                    else:
                        osb = sb.tile([64, NB], F32, tag="osb", bufs=3)
                        nc.vector.tensor_copy(osb[:], oa[0:64, :])
                    rf = sb.tile([1, NB], F32, tag="rf", bufs=2)
                    nc.vector.reciprocal_approx_fast(out=rf[:], in_=cs[:])
                    rbs = sb.tile([64, NB], F32, tag="rbs", bufs=2)
                    nc.gpsimd.partition_broadcast(rbs[:], rf[:])
                    if h == 0:
                        mdst = stk[0:64, nb * NB : (nb + 1) * NB]
                    elif h == 1:
                        mdst = outT1[0:64, nb * NB : (nb + 1) * NB]
                    else:
                        mdst = outT2[0:64, nb * NB : (nb + 1) * NB]
                    nc.vector.tensor_tensor(out=mdst, in0=osb[:], in1=rbs[:], op=MULT)
                    if h == 1:
                        nc.sync.dma_start(
                            stk[64:128, nb * NB : (nb + 1) * NB],
                            outT1[0:64, nb * NB : (nb + 1) * NB],
                        )
                    if h == HPC - 1:
                        for k in range(nb * 8, nb * 8 + 8):
                            bg.append((960, proj_item(k), True))

                return state, escores, eoa, epilogue

            for h in range(HPC):
                for nb in range(N // NB):
                    state, escores, eoa, epilogue = make_block(h, nb)
                    for g in range(MC // 2):
                        m = 2 * g
                        carried = bool(carry)
                        pump(
                            200 if carried else (700 if m > LAG else 1500),
                            pad=not carried,
                            allow_proj=(g >= 3),
                        )
                        escores(m)
                        escores(m + 1)
                        if carried:
                            carry.popleft()()
                        if m == LAG:
                            state["oa"] = ps.tile(
                                [65, NB], F32, tag="oa", bufs=1, name=f"oa{h}_{nb}"
                            )
                        if m >= LAG:
                            eoa(m - LAG)
                            eoa(m - LAG + 1)

                    def drain1(eoa=eoa):
                        eoa(MC - LAG)
                        eoa(MC - LAG + 1)

                    def drain2(eoa=eoa):
                        eoa(MC - LAG + 2)
                        eoa(MC - LAG + 3)

                    carry.append(drain1)
                    carry.append(drain2)
                    carry.append(epilogue)

            while carry:
                carry.popleft()()
                emit_pad(3)
            for _ in range(8):
                emit_pad(4)
            pump(10**9)

    nc.compile()
    return nc


def get_nc():
    global _NC_CACHE
    if _NC_CACHE is None:
        _NC_CACHE = build_nc()
    return _NC_CACHE


def make_in_maps(x, w_qkv, w_proj):
    """Shard inputs for the 8 cores: core c = 4*b + g."""
    in_maps = []
    for c in range(8):
        b, g = divmod(c, 4)
        heads = [3 * g + h for h in range(HPC)]
        xt = x[b].T.astype(ml_dtypes.bfloat16)  # [C, N]
        xt_p = np.ascontiguousarray(
            xt.reshape(KC, 128, N).transpose(1, 0, 2).reshape(128, KC * N)
        )
        wqk = np.empty((C, HPC * 128), dtype=ml_dtypes.bfloat16)
        wv = np.empty((C, HPC * D), dtype=ml_dtypes.bfloat16)
        for i, hh in enumerate(heads):
            wqk[:, i * 128 : i * 128 + 64] = w_qkv[:, hh * D : (hh + 1) * D]
            wqk[:, i * 128 + 64 : i * 128 + 128] = w_qkv[
                :, C + hh * D : C + (hh + 1) * D
            ]
            wv[:, i * D : (i + 1) * D] = w_qkv[:, 2 * C + hh * D : 2 * C + (hh + 1) * D]
        wqk_p = np.ascontiguousarray(
            wqk.reshape(KC, 128, HPC * 128).transpose(1, 0, 2).reshape(128, -1)
        )
        wv_p = np.ascontiguousarray(
            wv.reshape(KC, 128, HPC * D).transpose(1, 0, 2).reshape(128, -1)
        )
        wp = np.ascontiguousarray(w_proj[g * HPC * D : (g + 1) * HPC * D, :]).astype(
            ml_dtypes.bfloat16
        )
        in_maps.append({"xt": xt_p, "wqk": wqk_p, "wv": wv_p, "wp": wp})
    return in_maps


def run(x, w_qkv, w_proj, b_proj, trace=False):
    nc = get_nc()
    in_maps = make_in_maps(x, w_qkv, w_proj)
    res = run_bass_kernel_spmd(nc, in_maps, core_ids=list(range(8)), trace=trace)
    out = np.empty((B, N, C), dtype=np.float32)
    for b in range(B):
        acc = res.results[4 * b]["out"].astype(np.float32)
        for g in range(1, 4):
            acc = acc + res.results[4 * b + g]["out"]
        # unpack [128, NCH*C] -> [N, C]
        full = acc.reshape(128, NCH, C).transpose(1, 0, 2).reshape(N, C)
        out[b] = full + b_proj[None, :].astype(np.float32)
    return out, res


def kernel(x, w_qkv, w_proj, b_proj):
    out, _ = run(
        np.asarray(x), np.asarray(w_qkv), np.asarray(w_proj), np.asarray(b_proj)
    )
    return out


# revision 38
# speedup vs baseline: 1.1570x; 1.1570x over previous
"""Multi-head attention (B=2, N=2048, C=768, H=12) on 8 TRN2 NeuronCores.

Sharding: core c = 4*b + g handles batch b (data parallel) and heads
3g..3g+2 (tensor parallel on H). Each core computes its 3 heads end-to-end
plus the partial projection with its 192 rows of w_proj; the host sums the
4 partials per batch and adds b_proj. No cross-device communication.

Clock strategy: the PE reaches/holds 2.4 GHz only while its instruction
stream never waits on a semaphore; switching the PE tiling mode costs
~106ns per transition. Steady state is grouped so the PE always outpaces
ACT (the exp engine, ~1.04us per [128,1024] tile = the true floor) and
therefore never stalls:

  per 2-m-chunk group:  [scores m, m+1]   64-row-tile T0/T8 pairs (~432ns)
                        [attn@v m-3, m-2] full-K matmuls         (~930ns)
                        [one filler item]  full-K qk/v/proj piece (~300-600ns)
ACT does exp only; DVE does all PSUM evacuation + softmax normalization;
junk matmuls pad slots when no real filler remains.
"""

from collections import deque

import ml_dtypes
import numpy as np

import concourse.bass as bass
import concourse.mybir as mybir
import concourse.tile as tile
from concourse import bacc
from concourse.bass_utils import run_bass_kernel_spmd

F32 = mybir.dt.float32
F32R = mybir.dt.float32r
BF16 = mybir.dt.bfloat16
EXP = mybir.ActivationFunctionType.Exp
MULT = mybir.AluOpType.mult
ADD = mybir.AluOpType.add
I16 = mybir.dt.int16

B, N, C = 2, 2048, 768
H = 12
D = 64
HPC = 3  # heads per core
KC = 6  # contraction chunks of 128 over C
NB = 1024  # n-block for attention stage
MC = N // 128  # 16 m-chunks (context)
NCH = N // 128  # 16 output row chunks
SCALE = D ** -0.5
# Schraudolph-style integer exp constants for bf16 output bits
# (fold the 1/sqrt(D) attention scale into K1)
EXP_K1 = float(SCALE * 1.4426950408889634 * (1 << 7))
EXP_K2 = float((127 << 7) - 486411.0 / (1 << 16))
LAG = 4  # attn@v trails exp by this many m-chunks

_NC_CACHE = None


def build_nc():
    nc = bacc.Bacc("TRN2", target_bir_lowering=False, debug=False, num_devices=8)
    # host pre-packs every input into its exact SBUF layout so each DMA
    # moves long contiguous per-partition lines (4-24KB) at full HBM rate
    xt = nc.declare_dram_parameter("xt", [128, KC * N], BF16, isOutput=False)
    wqk = nc.declare_dram_parameter("wqk", [128, KC * HPC * 128], BF16, isOutput=False)
    wv = nc.declare_dram_parameter("wv", [128, KC * HPC * D], BF16, isOutput=False)
    wp = nc.declare_dram_parameter("wp", [HPC * D, C], BF16, isOutput=False)
    out = nc.declare_dram_parameter("out", [128, NCH * C], BF16, isOutput=True)

    with tile.TileContext(nc) as tc:
        with (
            tc.tile_pool(name="sb", bufs=1) as sb,
            tc.tile_pool(name="ps", bufs=1, space="PSUM") as ps,
            tc.tile_pool(name="drp", bufs=2, space="DRAM") as drp,
        ):
            # ---- input loads --------------------------------------------
            xtb = sb.tile([128, KC * N], BF16, tag="xtb")
            wqkb = sb.tile([128, KC * HPC * 128], BF16, tag="wqkb")
            wvb = sb.tile([128, KC * HPC * D], BF16, tag="wvb")
            # weights first (small), then x token-half by token-half so the
            # first attention block can start after ~half the load
            wp01 = sb.tile([128, C], BF16, tag="wp01")
            wp2 = sb.tile([64, C], BF16, tag="wp2")
            nc.sync.dma_start(wqkb[:], wqk[:])
            nc.scalar.dma_start(wvb[:], wv[:])
            for th in range(2):
                for kc in range(KC):
                    for pg in range(2):
                        sl = slice(pg * 64, (pg + 1) * 64)
                        eng = nc.sync if (kc + pg) % 2 else nc.scalar
                        eng.dma_start(
                            xtb[sl, kc * N + th * NB : kc * N + (th + 1) * NB],
                            xt[sl, kc * N + th * NB : kc * N + (th + 1) * NB],
                        )
            nc.sync.dma_start(wp01[:], wp[0:128, :])
            nc.sync.dma_start(wp2[:], wp[128 : HPC * D, :])

            junk = sb.tile([128, 512], BF16, tag="junk")
            nc.vector.memset(junk[:], 1.0)

            ones_f = sb.tile([128, MC], F32, tag="ones_f")
            nc.vector.memset(ones_f[:], 1.0)
            v_sb = sb.tile([128, HPC * MC * 65], BF16, tag="v")
            v4 = v_sb.rearrange("p (h m w) -> p h m w", h=HPC, m=MC)
            for h in range(HPC):
                nc.vector.tensor_copy(v4[:, h, :, 64], ones_f[:, :])

            qk_sb = [
                sb.tile([128, N], BF16, tag=f"qk{h}", name=f"qk{h}") for h in range(HPC)
            ]
            kq_sb = [
                sb.tile([128, N], BF16, tag=f"kq{h}", name=f"kq{h}") for h in range(HPC)
            ]
            stk = sb.tile([128, N], BF16, tag="stk")
            outT1 = sb.tile([64, N], BF16, tag="outT1")
            outT2 = sb.tile([64, N], BF16, tag="outT2")

            def sc_tile(name):
                return ps.tile([128, NB], F32, tag="sc", bufs=3, name=name)

            # ---- lead-in (full mode): ramp + qk head 0 ------------------
            for i in range(4):
                psw = sc_tile(f"junk{i}")
                nc.tensor.matmul(
                    psw[:, 0:512], junk[:, 0:128], junk[:], start=True, stop=True
                )
                nc.tensor.matmul(
                    psw[:, 512:1024], junk[:, 0:128], junk[:], start=True, stop=True
                )

            def emit_qk_half(h, half, psq, kc):
                hb = half * 1024
                for s in range(2):
                    nc.tensor.matmul(
                        psq[:, s * 512 : (s + 1) * 512],
                        wqkb[
                            :, kc * HPC * 128 + h * 128 : kc * HPC * 128 + (h + 1) * 128
                        ],
                        xtb[:, kc * N + hb + s * 512 : kc * N + hb + (s + 1) * 512],
                        start=(kc == 0),
                        stop=(kc == KC - 1),
                    )

            def emit_qk_tail(h, half, psq):
                hb = half * 1024
                nc.vector.tensor_copy(qk_sb[h][:, hb : hb + 1024], psq[:])
                eng = nc.scalar if h == 0 else nc.sync
                eng.dma_start(
                    kq_sb[h][0:64, hb : hb + 1024], qk_sb[h][64:128, hb : hb + 1024]
                )
                eng.dma_start(
                    kq_sb[h][64:128, hb : hb + 1024], qk_sb[h][0:64, hb : hb + 1024]
                )

            for half in range(2):
                psq = sc_tile(f"qk0h{half}")
                for kc in range(KC):
                    emit_qk_half(0, half, psq, kc)
                emit_qk_tail(0, half, psq)

            # ---- background PE work (full mode) -------------------------
            bg = deque()

            def v_item(m):
                def emit():
                    psv = sc_tile(f"v{m}")
                    for kc in range(KC):
                        nc.tensor.matmul(
                            psv[:, 0 : HPC * D],
                            xtb[:, kc * N + m * 128 : kc * N + (m + 1) * 128],
                            wvb[:, kc * HPC * D : (kc + 1) * HPC * D],
                            start=(kc == 0),
                            stop=(kc == KC - 1),
                        )
                    nc.vector.tensor_copy(
                        v4[:, :, m, 0:64],
                        psv[:, 0 : HPC * D].rearrange("p (h d) -> p h d", h=HPC),
                    )

                return emit

            def qk_item(h, half, state, step):
                # step 0..KC-1: one kc contraction step (2 matmuls ~432ns);
                # step KC: evacuation copy + partition swap (no PE work)
                def emit():
                    if step == 0:
                        state["ps"] = sc_tile(f"qk{h}h{half}")
                    if step < KC:
                        emit_qk_half(h, half, state["ps"], step)
                    else:
                        emit_qk_tail(h, half, state["ps"])

                return emit

            ob_state = {}

            def proj_item(k):
                def emit():
                    pp = sc_tile(f"pp{k}")
                    for sw, w in ((0, 512), (512, 256)):
                        nc.tensor.matmul(
                            pp[:, sw : sw + w],
                            stk[:, k * 128 : (k + 1) * 128],
                            wp01[:, sw : sw + w],
                            start=True,
                            stop=False,
                        )
                    for sw, w in ((0, 512), (512, 256)):
                        nc.tensor.matmul(
                            pp[:, sw : sw + w],
                            outT2[0:64, k * 128 : (k + 1) * 128],
                            wp2[:, sw : sw + w],
                            start=False,
                            stop=True,
                        )
                    if k % 2 == 0:
                        ob_state["t"] = sb.tile(
                            [128, 2 * C], BF16, tag="ob", bufs=2, name=f"ob{k}"
                        )
                    ob = ob_state["t"]
                    half = (k % 2) * C
                    if k >= 8:
                        # tail chunks: ACT is idle, split the evacuation
                        nc.vector.tensor_copy(ob[:, half : half + 384], pp[:, 0:384])
                        nc.scalar.copy(out=ob[:, half + 384 : half + C], in_=pp[:, 384:C])
                    else:
                        nc.vector.tensor_copy(ob[:, half : half + C], pp[:, 0:C])
                    if k % 2 == 1:
                        eng = nc.scalar if k >= 8 and (k // 2) % 2 else nc.sync
                        eng.dma_start(out[:, (k - 1) * C : (k + 1) * C], ob[:])

                return emit

            for m in range(MC):
                bg.append((640, v_item(m), False))
            for h in (1, 2):
                for half in range(2):
                    state = {}
                    for step in range(KC + 1):
                        bg.append(
                            (440 if step < KC else 0, qk_item(h, half, state, step), False)
                        )

            pad_idx = [0]

            def emit_pad(n_mm):
                psw = sc_tile(f"pad{pad_idx[0]}")
                pad_idx[0] += 1
                for s in range(n_mm):
                    nc.tensor.matmul(
                        psw[:, (s % 2) * 512 : (s % 2 + 1) * 512],
                        junk[:, 0:128],
                        junk[:],
                        start=True,
                        stop=True,
                    )

            def pump(budget, pad=False, allow_proj=True):
                # consume bg items until the PE-time budget is spent; if the
                # queue is dry, emit ONE junk pad tile to fill the remainder
                while bg and budget > 0:
                    if bg[0][2] and not allow_proj:
                        break
                    cost, emit, _ = bg.popleft()
                    emit()
                    budget -= cost
                if pad and budget > 200:
                    emit_pad(2)

            # ---- attention: blocks software-pipelined across boundaries --
            # The last two oa groups + the normalization epilogue of block i
            # run during the first groups of block i+1 so ACT never starves
            # at a boundary waiting for the PE to sprint the drain.
            carry = deque()

            def make_block(h, nb):
                state = {"ex": {}, "oa": None}

                def escores(m):
                    sc = sc_tile(f"sc{h}_{nb}_{m}")
                    nc.tensor.matmul(
                        sc[:, 0:512],
                        kq_sb[h][0:64, m * 128 : (m + 1) * 128],
                        qk_sb[h][0:64, nb * NB : nb * NB + 512],
                        start=True,
                        stop=True,
                        tile_position=(0, 0),
                    )
                    nc.tensor.matmul(
                        sc[:, 512:1024],
                        qk_sb[h][64:128, m * 128 : (m + 1) * 128],
                        kq_sb[h][64:128, nb * NB + 512 : nb * NB + 1024],
                        start=True,
                        stop=True,
                        tile_position=(64, 0),
                    )
                    ex = sb.tile([128, NB], BF16, tag="ex", bufs=9)
                    if m % 5 == 4:
                        # offload to DVE: exp(x) ~ bitcast(int32(x*K1 + K2));
                        # end-to-end rel err stays < 1e-2 (verified in sim)
                        nc.vector.tensor_scalar(
                            out=ex[:].bitcast(I16),
                            in0=sc[:],
                            scalar1=EXP_K1,
                            scalar2=EXP_K2,
                            op0=MULT,
                            op1=ADD,
                        )
                    else:
                        nc.scalar.activation(ex[:], sc[:], EXP, scale=SCALE)
                    state["ex"][m] = ex

                def eoa(m):
                    exm = state["ex"].pop(m)
                    for s in range(2):
                        nc.tensor.matmul(
                            state["oa"][:, s * 512 : (s + 1) * 512],
                            v4[:, h, m, :],
                            exm[:, s * 512 : (s + 1) * 512],
                            start=(m == 0),
                            stop=(m == MC - 1),
                        )

                def epilogue():
                    final = h == HPC - 1 and nb == 1
                    oa = state["oa"]
                    cs = sb.tile([1, NB], F32, tag="cs", bufs=2)
                    nc.vector.tensor_copy(cs[:], oa[64:65, :])
                    if final:
                        osb = oa[0:64, :]
                    else:
                        osb = sb.tile([64, NB], F32, tag="osb", bufs=3)
                        nc.vector.tensor_copy(osb[:], oa[0:64, :])
                    rf = sb.tile([1, NB], F32, tag="rf", bufs=2)
                    nc.vector.reciprocal_approx_fast(out=rf[:], in_=cs[:])
                    rbs = sb.tile([64, NB], F32, tag="rbs", bufs=2)
                    nc.gpsimd.partition_broadcast(rbs[:], rf[:])
                    if h == 0:
                        mdst = stk[0:64, nb * NB : (nb + 1) * NB]
                    elif h == 1:
                        mdst = outT1[0:64, nb * NB : (nb + 1) * NB]
                    else:
                        mdst = outT2[0:64, nb * NB : (nb + 1) * NB]
                    nc.vector.tensor_tensor(out=mdst, in0=osb[:], in1=rbs[:], op=MULT)
                    if h == 1:
                        nc.sync.dma_start(
                            stk[64:128, nb * NB : (nb + 1) * NB],
                            outT1[0:64, nb * NB : (nb + 1) * NB],
                        )
                    if h == HPC - 1:
                        for k in range(nb * 8, nb * 8 + 8):
                            bg.append((960, proj_item(k), True))

                return state, escores, eoa, epilogue

            for h in range(HPC):
                for nb in range(N // NB):
                    state, escores, eoa, epilogue = make_block(h, nb)
                    for g in range(MC // 2):
                        m = 2 * g
                        carried = bool(carry)
                        pump(
                            200 if carried else (950 if m > LAG else 1600),
                            pad=not carried,
                            allow_proj=(g >= 3),
                        )
                        escores(m)
                        escores(m + 1)
                        if carried:
                            carry.popleft()()
                        if m == LAG:
                            state["oa"] = ps.tile(
                                [65, NB], F32, tag="oa", bufs=1, name=f"oa{h}_{nb}"
                            )
                        if m >= LAG:
                            eoa(m - LAG)
                            eoa(m - LAG + 1)

                    def drain1(eoa=eoa):
                        eoa(MC - LAG)
                        eoa(MC - LAG + 1)

                    def drain2(eoa=eoa):
                        eoa(MC - LAG + 2)
                        eoa(MC - LAG + 3)

                    carry.append(drain1)
                    carry.append(drain2)
                    carry.append(epilogue)

            while carry:
                carry.popleft()()
                emit_pad(3)
            for _ in range(8):
                emit_pad(4)
            pump(10**9)

    nc.compile()
    return nc


def get_nc():
    global _NC_CACHE
    if _NC_CACHE is None:
        _NC_CACHE = build_nc()
    return _NC_CACHE


def make_in_maps(x, w_qkv, w_proj):
    """Shard inputs for the 8 cores: core c = 4*b + g."""
    in_maps = []
    for c in range(8):
        b, g = divmod(c, 4)
        heads = [3 * g + h for h in range(HPC)]
        xt = x[b].T.astype(ml_dtypes.bfloat16)  # [C, N]
        xt_p = np.ascontiguousarray(
            xt.reshape(KC, 128, N).transpose(1, 0, 2).reshape(128, KC * N)
        )
        wqk = np.empty((C, HPC * 128), dtype=ml_dtypes.bfloat16)
        wv = np.empty((C, HPC * D), dtype=ml_dtypes.bfloat16)
        for i, hh in enumerate(heads):
            wqk[:, i * 128 : i * 128 + 64] = w_qkv[:, hh * D : (hh + 1) * D]
            wqk[:, i * 128 + 64 : i * 128 + 128] = w_qkv[
                :, C + hh * D : C + (hh + 1) * D
            ]
            wv[:, i * D : (i + 1) * D] = w_qkv[:, 2 * C + hh * D : 2 * C + (hh + 1) * D]
        wqk_p = np.ascontiguousarray(
            wqk.reshape(KC, 128, HPC * 128).transpose(1, 0, 2).reshape(128, -1)
        )
        wv_p = np.ascontiguousarray(
            wv.reshape(KC, 128, HPC * D).transpose(1, 0, 2).reshape(128, -1)
        )
        wp = np.ascontiguousarray(w_proj[g * HPC * D : (g + 1) * HPC * D, :]).astype(
            ml_dtypes.bfloat16
        )
        in_maps.append({"xt": xt_p, "wqk": wqk_p, "wv": wv_p, "wp": wp})
    return in_maps


def run(x, w_qkv, w_proj, b_proj, trace=False):
    nc = get_nc()
    in_maps = make_in_maps(x, w_qkv, w_proj)
    res = run_bass_kernel_spmd(nc, in_maps, core_ids=list(range(8)), trace=trace)
    out = np.empty((B, N, C), dtype=np.float32)
    for b in range(B):
        acc = res.results[4 * b]["out"].astype(np.float32)
        for g in range(1, 4):
            acc = acc + res.results[4 * b + g]["out"]
        # unpack [128, NCH*C] -> [N, C]
        full = acc.reshape(128, NCH, C).transpose(1, 0, 2).reshape(N, C)
        out[b] = full + b_proj[None, :].astype(np.float32)
    return out, res


def kernel(x, w_qkv, w_proj, b_proj):
    out, _ = run(
        np.asarray(x), np.asarray(w_qkv), np.asarray(w_proj), np.asarray(b_proj)
    )
    return out


# revision 39
# speedup vs baseline: 1.2012x; 1.0382x over previous
"""Multi-head attention (B=2, N=2048, C=768, H=12) on 8 TRN2 NeuronCores.

Sharding: core c = 4*b + g handles batch b (data parallel) and heads
3g..3g+2 (tensor parallel on H). Each core computes its 3 heads end-to-end
plus the partial projection with its 192 rows of w_proj; the host sums the
4 partials per batch and adds b_proj. No cross-device communication.

Clock strategy: the PE reaches/holds 2.4 GHz only while its instruction
stream never waits on a semaphore; switching the PE tiling mode costs
~106ns per transition. Steady state is grouped so the PE always outpaces
ACT (the exp engine, ~1.04us per [128,1024] tile = the true floor) and
therefore never stalls:

  per 2-m-chunk group:  [scores m, m+1]   64-row-tile T0/T8 pairs (~432ns)
                        [attn@v m-3, m-2] full-K matmuls         (~930ns)
                        [one filler item]  full-K qk/v/proj piece (~300-600ns)
ACT does exp only; DVE does all PSUM evacuation + softmax normalization;
junk matmuls pad slots when no real filler remains.
"""

from collections import deque

import ml_dtypes
import numpy as np

import concourse.bass as bass
import concourse.mybir as mybir
import concourse.tile as tile
from concourse import bacc
from concourse.bass_utils import run_bass_kernel_spmd

F32 = mybir.dt.float32
F32R = mybir.dt.float32r
BF16 = mybir.dt.bfloat16
EXP = mybir.ActivationFunctionType.Exp
MULT = mybir.AluOpType.mult
ADD = mybir.AluOpType.add
I16 = mybir.dt.int16

B, N, C = 2, 2048, 768
H = 12
D = 64
HPC = 3  # heads per core
KC = 6  # contraction chunks of 128 over C
NB = 1024  # n-block for attention stage
MC = N // 128  # 16 m-chunks (context)
NCH = N // 128  # 16 output row chunks
SCALE = D ** -0.5
# Schraudolph-style integer exp constants for bf16 output bits
# (fold the 1/sqrt(D) attention scale into K1)
EXP_K1 = float(SCALE * 1.4426950408889634 * (1 << 7))
EXP_K2 = float((127 << 7) - 486411.0 / (1 << 16))
LAG = 4  # attn@v trails exp by this many m-chunks

_NC_CACHE = None


def build_nc():
    nc = bacc.Bacc("TRN2", target_bir_lowering=False, debug=False, num_devices=8)
    # host pre-packs every input into its exact SBUF layout so each DMA
    # moves long contiguous per-partition lines (4-24KB) at full HBM rate
    xt = nc.declare_dram_parameter("xt", [128, KC * N], BF16, isOutput=False)
    wqk = nc.declare_dram_parameter("wqk", [128, KC * HPC * 128], BF16, isOutput=False)
    wv = nc.declare_dram_parameter("wv", [128, KC * HPC * D], BF16, isOutput=False)
    wp = nc.declare_dram_parameter("wp", [HPC * D, C], BF16, isOutput=False)
    out = nc.declare_dram_parameter("out", [128, NCH * C], BF16, isOutput=True)

    with tile.TileContext(nc) as tc:
        with (
            tc.tile_pool(name="sb", bufs=1) as sb,
            tc.tile_pool(name="ps", bufs=1, space="PSUM") as ps,
            tc.tile_pool(name="drp", bufs=2, space="DRAM") as drp,
        ):
            # ---- input loads --------------------------------------------
            xtb = sb.tile([128, KC * N], BF16, tag="xtb")
            wqkb = sb.tile([128, KC * HPC * 128], BF16, tag="wqkb")
            wvb = sb.tile([128, KC * HPC * D], BF16, tag="wvb")
            # weights first (small), then x token-half by token-half so the
            # first attention block can start after ~half the load
            wp01 = sb.tile([128, C], BF16, tag="wp01")
            wp2 = sb.tile([64, C], BF16, tag="wp2")
            nc.sync.dma_start(wqkb[:], wqk[:])
            nc.scalar.dma_start(wvb[:], wv[:])
            for th in range(2):
                for kc in range(KC):
                    for pg in range(2):
                        sl = slice(pg * 64, (pg + 1) * 64)
                        eng = nc.sync if (kc + pg) % 2 else nc.scalar
                        eng.dma_start(
                            xtb[sl, kc * N + th * NB : kc * N + (th + 1) * NB],
                            xt[sl, kc * N + th * NB : kc * N + (th + 1) * NB],
                        )
            nc.sync.dma_start(wp01[:], wp[0:128, :])
            nc.sync.dma_start(wp2[:], wp[128 : HPC * D, :])

            junk = sb.tile([128, 512], BF16, tag="junk")
            nc.vector.memset(junk[:], 1.0)

            ones_f = sb.tile([128, MC], F32, tag="ones_f")
            nc.vector.memset(ones_f[:], 1.0)
            v_sb = sb.tile([128, HPC * MC * 65], BF16, tag="v")
            v4 = v_sb.rearrange("p (h m w) -> p h m w", h=HPC, m=MC)
            for h in range(HPC):
                nc.vector.tensor_copy(v4[:, h, :, 64], ones_f[:, :])

            qk_sb = [
                sb.tile([128, N], BF16, tag=f"qk{h}", name=f"qk{h}") for h in range(HPC)
            ]
            kq_sb = [
                sb.tile([128, N], BF16, tag=f"kq{h}", name=f"kq{h}") for h in range(HPC)
            ]
            stk = sb.tile([128, N], BF16, tag="stk")
            outT1 = sb.tile([64, N], BF16, tag="outT1")
            outT2 = sb.tile([64, N], BF16, tag="outT2")

            def sc_tile(name):
                return ps.tile([128, NB], F32, tag="sc", bufs=3, name=name)

            # ---- lead-in (full mode): ramp + qk head 0 ------------------
            for i in range(4):
                psw = sc_tile(f"junk{i}")
                nc.tensor.matmul(
                    psw[:, 0:512], junk[:, 0:128], junk[:], start=True, stop=True
                )
                nc.tensor.matmul(
                    psw[:, 512:1024], junk[:, 0:128], junk[:], start=True, stop=True
                )

            def emit_qk_half(h, half, psq, kc):
                hb = half * 1024
                for s in range(2):
                    nc.tensor.matmul(
                        psq[:, s * 512 : (s + 1) * 512],
                        wqkb[
                            :, kc * HPC * 128 + h * 128 : kc * HPC * 128 + (h + 1) * 128
                        ],
                        xtb[:, kc * N + hb + s * 512 : kc * N + hb + (s + 1) * 512],
                        start=(kc == 0),
                        stop=(kc == KC - 1),
                    )

            def emit_qk_tail(h, half, psq):
                hb = half * 1024
                if h == 0:
                    nc.vector.tensor_copy(qk_sb[h][:, hb : hb + 512], psq[:, 0:512])
                    nc.scalar.copy(
                        out=qk_sb[h][:, hb + 512 : hb + 1024], in_=psq[:, 512:1024]
                    )
                else:
                    nc.vector.tensor_copy(qk_sb[h][:, hb : hb + 1024], psq[:])
                eng = nc.scalar if h == 0 else nc.sync
                eng.dma_start(
                    kq_sb[h][0:64, hb : hb + 1024], qk_sb[h][64:128, hb : hb + 1024]
                )
                eng.dma_start(
                    kq_sb[h][64:128, hb : hb + 1024], qk_sb[h][0:64, hb : hb + 1024]
                )

            for half in range(2):
                psq = sc_tile(f"qk0h{half}")
                for kc in range(KC):
                    emit_qk_half(0, half, psq, kc)
                emit_qk_tail(0, half, psq)

            # ---- background PE work (full mode) -------------------------
            bg = deque()

            def v_item(m):
                def emit():
                    psv = sc_tile(f"v{m}")
                    for kc in range(KC):
                        nc.tensor.matmul(
                            psv[:, 0 : HPC * D],
                            xtb[:, kc * N + m * 128 : kc * N + (m + 1) * 128],
                            wvb[:, kc * HPC * D : (kc + 1) * HPC * D],
                            start=(kc == 0),
                            stop=(kc == KC - 1),
                        )
                    nc.vector.tensor_copy(
                        v4[:, :, m, 0:64],
                        psv[:, 0 : HPC * D].rearrange("p (h d) -> p h d", h=HPC),
                    )

                return emit

            def qk_item(h, half, state, step):
                # step 0..KC-1: one kc contraction step (2 matmuls ~432ns);
                # step KC: evacuation copy + partition swap (no PE work)
                def emit():
                    if step == 0:
                        state["ps"] = sc_tile(f"qk{h}h{half}")
                    if step < KC:
                        emit_qk_half(h, half, state["ps"], step)
                    else:
                        emit_qk_tail(h, half, state["ps"])

                return emit

            ob_state = {}

            def proj_item(k):
                def emit():
                    pp = sc_tile(f"pp{k}")
                    for sw, w in ((0, 512), (512, 256)):
                        nc.tensor.matmul(
                            pp[:, sw : sw + w],
                            stk[:, k * 128 : (k + 1) * 128],
                            wp01[:, sw : sw + w],
                            start=True,
                            stop=False,
                        )
                    for sw, w in ((0, 512), (512, 256)):
                        nc.tensor.matmul(
                            pp[:, sw : sw + w],
                            outT2[0:64, k * 128 : (k + 1) * 128],
                            wp2[:, sw : sw + w],
                            start=False,
                            stop=True,
                        )
                    if k % 2 == 0:
                        ob_state["t"] = sb.tile(
                            [128, 2 * C], BF16, tag="ob", bufs=3, name=f"ob{k}"
                        )
                    ob = ob_state["t"]
                    half = (k % 2) * C
                    if k >= 8:
                        # tail chunks: ACT is idle, split the evacuation
                        nc.vector.tensor_copy(ob[:, half : half + 384], pp[:, 0:384])
                        nc.scalar.copy(out=ob[:, half + 384 : half + C], in_=pp[:, 384:C])
                    else:
                        nc.vector.tensor_copy(ob[:, half : half + C], pp[:, 0:C])
                    if k % 2 == 1:
                        eng = nc.scalar if k >= 8 and (k // 2) % 2 else nc.sync
                        eng.dma_start(out[:, (k - 1) * C : (k + 1) * C], ob[:])

                return emit

            for m in range(MC):
                bg.append((640, v_item(m), False))
            for h in (1, 2):
                for half in range(2):
                    state = {}
                    for step in range(KC + 1):
                        bg.append(
                            (440 if step < KC else 0, qk_item(h, half, state, step), False)
                        )

            pad_idx = [0]

            def emit_pad(n_mm):
                psw = sc_tile(f"pad{pad_idx[0]}")
                pad_idx[0] += 1
                for s in range(n_mm):
                    nc.tensor.matmul(
                        psw[:, (s % 2) * 512 : (s % 2 + 1) * 512],
                        junk[:, 0:128],
                        junk[:],
                        start=True,
                        stop=True,
                    )

            def pump(budget, pad=False, allow_proj=True):
                # consume bg items until the PE-time budget is spent; if the
                # queue is dry, emit ONE junk pad tile to fill the remainder
                while bg and budget > 0:
                    if bg[0][2] and not allow_proj:
                        break
                    cost, emit, _ = bg.popleft()
                    emit()
                    budget -= cost
                if pad and budget > 200:
                    emit_pad(2)

            # ---- attention: blocks software-pipelined across boundaries --
            # The last two oa groups + the normalization epilogue of block i
            # run during the first groups of block i+1 so ACT never starves
            # at a boundary waiting for the PE to sprint the drain.
            carry = deque()

            def make_block(h, nb):
                state = {"ex": {}, "oa": None}

                def escores(m):
                    sc = sc_tile(f"sc{h}_{nb}_{m}")
                    nc.tensor.matmul(
                        sc[:, 0:512],
                        kq_sb[h][0:64, m * 128 : (m + 1) * 128],
                        qk_sb[h][0:64, nb * NB : nb * NB + 512],
                        start=True,
                        stop=True,
                        tile_position=(0, 0),
                    )
                    nc.tensor.matmul(
                        sc[:, 512:1024],
                        qk_sb[h][64:128, m * 128 : (m + 1) * 128],
                        kq_sb[h][64:128, nb * NB + 512 : nb * NB + 1024],
                        start=True,
                        stop=True,
                        tile_position=(64, 0),
                    )
                    ex = sb.tile([128, NB], BF16, tag="ex", bufs=9)
                    if m % 5 == 4:
                        # offload to DVE: exp(x) ~ bitcast(int32(x*K1 + K2));
                        # end-to-end rel err stays < 1e-2 (verified in sim)
                        nc.vector.tensor_scalar(
                            out=ex[:].bitcast(I16),
                            in0=sc[:],
                            scalar1=EXP_K1,
                            scalar2=EXP_K2,
                            op0=MULT,
                            op1=ADD,
                        )
                    else:
                        nc.scalar.activation(ex[:], sc[:], EXP, scale=SCALE)
                    state["ex"][m] = ex

                def eoa(m):
                    exm = state["ex"].pop(m)
                    for s in range(2):
                        nc.tensor.matmul(
                            state["oa"][:, s * 512 : (s + 1) * 512],
                            v4[:, h, m, :],
                            exm[:, s * 512 : (s + 1) * 512],
                            start=(m == 0),
                            stop=(m == MC - 1),
                        )

                def epilogue():
                    final = h == HPC - 1 and nb == 1
                    oa = state["oa"]
                    cs = sb.tile([1, NB], F32, tag="cs", bufs=2)
                    nc.vector.tensor_copy(cs[:], oa[64:65, :])
                    if final:
                        osb = oa[0:64, :]
                    else:
                        osb = sb.tile([64, NB], F32, tag="osb", bufs=3)
                        nc.vector.tensor_copy(osb[:], oa[0:64, :])
                    rf = sb.tile([1, NB], F32, tag="rf", bufs=2)
                    nc.vector.reciprocal_approx_fast(out=rf[:], in_=cs[:])
                    rbs = sb.tile([64, NB], F32, tag="rbs", bufs=2)
                    nc.gpsimd.partition_broadcast(rbs[:], rf[:])
                    if h == 0:
                        mdst = stk[0:64, nb * NB : (nb + 1) * NB]
                    elif h == 1:
                        mdst = outT1[0:64, nb * NB : (nb + 1) * NB]
                    else:
                        mdst = outT2[0:64, nb * NB : (nb + 1) * NB]
                    nc.vector.tensor_tensor(out=mdst, in0=osb[:], in1=rbs[:], op=MULT)
                    if h == 1:
                        nc.sync.dma_start(
                            stk[64:128, nb * NB : (nb + 1) * NB],
                            outT1[0:64, nb * NB : (nb + 1) * NB],
                        )
                    if h == HPC - 1:
                        for k in range(nb * 8, nb * 8 + 8):
                            bg.append((960, proj_item(k), True))

                return state, escores, eoa, epilogue

            for h in range(HPC):
                for nb in range(N // NB):
                    state, escores, eoa, epilogue = make_block(h, nb)
                    for g in range(MC // 2):
                        m = 2 * g
                        carried = bool(carry)
                        pump(
                            200 if carried else (950 if m > LAG else 1600),
                            pad=not carried,
                            allow_proj=(g >= 3),
                        )
                        escores(m)
                        escores(m + 1)
                        if carried:
                            carry.popleft()()
                        if m == LAG:
                            state["oa"] = ps.tile(
                                [65, NB], F32, tag="oa", bufs=1, name=f"oa{h}_{nb}"
                            )
                        if m >= LAG:
                            eoa(m - LAG)
                            eoa(m - LAG + 1)

                    def drain1(eoa=eoa):
                        eoa(MC - LAG)
                        eoa(MC - LAG + 1)

                    def drain2(eoa=eoa):
                        eoa(MC - LAG + 2)
                        eoa(MC - LAG + 3)

                    carry.append(drain1)
                    carry.append(drain2)
                    carry.append(epilogue)

            while carry:
                carry.popleft()()
                emit_pad(3)
            for _ in range(8):
                emit_pad(4)
            pump(10**9)

    nc.compile()
    return nc


def get_nc():
    global _NC_CACHE
    if _NC_CACHE is None:
        _NC_CACHE = build_nc()
    return _NC_CACHE


def make_in_maps(x, w_qkv, w_proj):
    """Shard inputs for the 8 cores: core c = 4*b + g."""
    in_maps = []
    for c in range(8):
        b, g = divmod(c, 4)
        heads = [3 * g + h for h in range(HPC)]
        xt = x[b].T.astype(ml_dtypes.bfloat16)  # [C, N]
        xt_p = np.ascontiguousarray(
            xt.reshape(KC, 128, N).transpose(1, 0, 2).reshape(128, KC * N)
        )
        wqk = np.empty((C, HPC * 128), dtype=ml_dtypes.bfloat16)
        wv = np.empty((C, HPC * D), dtype=ml_dtypes.bfloat16)
        for i, hh in enumerate(heads):
            wqk[:, i * 128 : i * 128 + 64] = w_qkv[:, hh * D : (hh + 1) * D]
            wqk[:, i * 128 + 64 : i * 128 + 128] = w_qkv[
                :, C + hh * D : C + (hh + 1) * D
            ]
            wv[:, i * D : (i + 1) * D] = w_qkv[:, 2 * C + hh * D : 2 * C + (hh + 1) * D]
        wqk_p = np.ascontiguousarray(
            wqk.reshape(KC, 128, HPC * 128).transpose(1, 0, 2).reshape(128, -1)
        )
        wv_p = np.ascontiguousarray(
            wv.reshape(KC, 128, HPC * D).transpose(1, 0, 2).reshape(128, -1)
        )
        wp = np.ascontiguousarray(w_proj[g * HPC * D : (g + 1) * HPC * D, :]).astype(
            ml_dtypes.bfloat16
        )
        in_maps.append({"xt": xt_p, "wqk": wqk_p, "wv": wv_p, "wp": wp})
    return in_maps


def run(x, w_qkv, w_proj, b_proj, trace=False):
    nc = get_nc()
    in_maps = make_in_maps(x, w_qkv, w_proj)
    res = run_bass_kernel_spmd(nc, in_maps, core_ids=list(range(8)), trace=trace)
    out = np.empty((B, N, C), dtype=np.float32)
    for b in range(B):
        acc = res.results[4 * b]["out"].astype(np.float32)
        for g in range(1, 4):
            acc = acc + res.results[4 * b + g]["out"]
        # unpack [128, NCH*C] -> [N, C]
        full = acc.reshape(128, NCH, C).transpose(1, 0, 2).reshape(N, C)
        out[b] = full + b_proj[None, :].astype(np.float32)
    return out, res


def kernel(x, w_qkv, w_proj, b_proj):
    out, _ = run(
        np.asarray(x), np.asarray(w_qkv), np.asarray(w_proj), np.asarray(b_proj)
    )
    return out
